# revision 48
# baseline (speedup 1.0000x reference)
"""
Causal ALiBi GQA attention (B=1, S=4096, D=1024, H=16, KVH=4, dh=64) on 8
Trainium2 NeuronCores via Bass/Tile.

Sharding: head-parallel with ALiBi-band load balancing. Core c handles
  - head A = 8+c (small ALiBi slope -> full causal window), and
  - head B = 7-c (large slope -> only the last 6 key-tiles per query chunk
    matter; dropped keys contribute < 1e-9 relative).
Every core therefore runs the identical instruction schedule (SPMD), while
all per-core identity (which heads / kv-heads / slopes) lives in the input
arrays. The 8 partial [S,D] outputs are summed on the host (the unshard).

Device layout (per core), fp32 storage with float32r (single-pass PE,
4x faster than fp32's hi/lo 2-pass) matmuls everywhere:
  - qkv arrives pre-transposed from the host: qkv_t [D, S] (D on
    partitions), DMA'd straight into the matmul operand tiles (PE reads
    the fp32 bits in reduced precision; no pre-round pass needed).
  - Q/K projections emit both heads stacked on 128 partitions (A on 0:64,
    B on 64:128) from one matmul chain; V is projected the same way and
    PE-transposed in [128,128] blocks feeding both heads' V' tiles.
  - Head A: pure q.k fp32r, contraction 64; its alibi enters as an exact
    per-(k-tile, q-chunk) fp32 ACT bias slope_A*(k - q_max(qc)) on the
    exp. The induced per-q factor exp(slope_A*(q - q_max)) cancels in the
    softmax division and stays in fp32 range because slope_A <= 0.075.
  - Head B: pure q.k fp32r as well; its alibi + causal mask come from six
    precomputed [128,512] bias tables indexed by the tile diagonal offset
    a = kt - 4*qc (exact fp32 DVE add before the exp). Logits <= ~3, so
    no running max is needed.
  - Causal mask for head A: -1e30 added on diagonal blocks before exp.
  - k-tiles are processed in pairs sharing one [128,1024] PSUM tile so
    mask/bias adds and head-B exps run at 1024 width.
  - V'_g [128 kpos, 68]: cols 0:64 = V, cols 64:68 = 1.0; PV accumulates
    O' [68, 512q] whose rows 64:68 hold the softmax denominator d[q]. A
    contraction-4 matmul broadcasts d across partitions; after an approx
    reciprocal (18-bit, ample for the 2e-2 gate) + multiply, the two
    normalized heads are stacked [128, 512] so the output projection runs
    with a full 128-deep contraction. PSUM->SBUF output copies run on the
    otherwise-idle Pool engine.
"""

import os
import sys
from contextlib import ExitStack

sys.path.insert(0, "/opt/trn_rl_repo")

import numpy as np

import concourse.bass as bass
import concourse.mybir as mybir
import concourse.tile as tile
from concourse import bass2jax as _bass2jax
from concourse import bass_utils as _bass_utils
from concourse.bass_utils import run_bass_kernel_spmd


def _legalize_bir_sync(bir_json):
    """The TPB ISA embeds at most ONE semaphore wait per instruction
    (NEURON_ISA_TPB_EVENTS has a single wait slot), and this walrus build
    refuses instructions carrying more ("Too many sync wait commands")
    instead of splitting them. Tile attaches up to ~11 waits to one
    instruction, so hoist all but the last wait onto standalone
    EventSemaphore instructions (the exact form raw-bass wait_ge emits)
    immediately before the instruction in its engine stream."""
    import json as _json
    d = _json.loads(bir_json)
    n = 0
    for f in d.get("functions", []):
        for b in f.get("blocks", []):
            insts = b.get("instructions")
            if not insts:
                continue
            out = []
            changed = False
            for i in insts:
                si = i.get("sync_info")
                if si:
                    w = si.get("on_wait") or []
                    u = si.get("on_update") or []
                    assert len(u) <= 1, f"multi-update on {i.get('name')}"
                    if len(w) > 1:
                        changed = True
                        for extra in w[:-1]:
                            n += 1
                            out.append({
                                "debug": i.get("debug", 0),
                                "engine": i["engine"],
                                "ins": [], "outs": [],
                                "name": f"I-legw{n}",
                                "opcode": "EventSemaphore",
                                "sync_info": {"on_update": [],
                                              "on_wait": [extra]},
                            })
                        si["on_wait"] = [w[-1]]
                out.append(i)
            if changed:
                b["instructions"] = out
    return _json.dumps(d).encode()


_ORIG_COMPILE_BIR = _bass_utils.compile_bir_kernel


def _patched_compile_bir_kernel(bir_json, tmpdir, neff_name="file.neff"):
    return _ORIG_COMPILE_BIR(_legalize_bir_sync(bir_json), tmpdir, neff_name)


if _bass_utils.compile_bir_kernel is not _patched_compile_bir_kernel:
    _bass_utils.compile_bir_kernel = _patched_compile_bir_kernel
    _bass2jax.compile_bir_kernel = _patched_compile_bir_kernel

P = 128
DM = 1024
DH = 64
SCALE = 1.0 / 8.0  # 1/sqrt(dh)
NEG = -1.0e30
KB = 5  # banded head: key-tiles kept per query chunk; nearest dropped
        # key sits >= 128 positions behind the chunk, weight < 2e-6

LAST = {}
ABL = set()


def build_program(S, reps=1, rep_scope="all"):
    f32 = mybir.dt.float32
    f32r = mybir.dt.float32r

    def r(ap):
        # single-pass reduced-precision PE multiply: 4x faster than fp32
        return ap.bitcast(f32r)
    KT_N = S // 128
    QC_N = S // 512

    nc = bass.Bass()
    qkv_t = nc.dram_tensor("qkv_t", [DM, S], f32, kind="ExternalInput")
    wq = nc.dram_tensor("wq", [DM, P], f32, kind="ExternalInput")
    wkv = nc.dram_tensor("wkv", [DM, 256], f32, kind="ExternalInput")
    wo = nc.dram_tensor("wo", [P, DM], f32, kind="ExternalInput")
    bq2 = nc.dram_tensor("bq2", [P, 1], f32, kind="ExternalInput")
    bkv2 = nc.dram_tensor("bkv2", [P, 2], f32, kind="ExternalInput")
    masks = nc.dram_tensor("masks", [P, 2048], f32, kind="ExternalInput")
    abias = nc.dram_tensor("abias", [P, 256], f32, kind="ExternalInput")
    # head-B alibi as a rank-2 matmul: bias[p,j] = c1[a][p] - slope*j,
    # hi/lo split so the f32r operand rounding stays exact to ~2^-22
    abr = nc.dram_tensor("abr", [4, KB * P], f32, kind="ExternalInput")
    abj = nc.dram_tensor("abj", [4, 512], f32, kind="ExternalInput")
    ident = nc.dram_tensor("ident", [P, P], f32, kind="ExternalInput")
    out = nc.dram_tensor("out", [S, DM], f32, kind="ExternalOutput")

    ExpF = mybir.ActivationFunctionType.Exp
    ADD = mybir.AluOpType.add
    MUL = mybir.AluOpType.mult

    QW = 1024 if S >= 1024 else S  # s-stream width for projections
    NHF = S // QW
    NCH = QW // 512

    with ExitStack() as ctx:
        tc = ctx.enter_context(tile.TileContext(nc))
        pers = ctx.enter_context(tc.tile_pool(name="pers", bufs=1))
        qkvp = ctx.enter_context(tc.tile_pool(name="qkvp", bufs=2))
        stg = ctx.enter_context(tc.tile_pool(name="stg", bufs=2))
        ptp = ctx.enter_context(tc.tile_pool(name="ptp", bufs=6))
        osbp = ctx.enter_context(tc.tile_pool(name="osbp", bufs=3))
        rrp = ctx.enter_context(tc.tile_pool(name="rrp", bufs=2))
        onp = ctx.enter_context(tc.tile_pool(name="onp", bufs=2))
        stkp = ctx.enter_context(tc.tile_pool(name="stkp", bufs=2))
        outp = ctx.enter_context(tc.tile_pool(name="outp", bufs=4))
        sps = ctx.enter_context(tc.tile_pool(name="sps", bufs=4, space="PSUM"))
        pop = ctx.enter_context(tc.tile_pool(name="pop", bufs=2, space="PSUM"))
        ops = ctx.enter_context(tc.tile_pool(name="ops", bufs=2, space="PSUM"))

        qq = pers.tile([P, S], f32, tag="qq")
        kk = pers.tile([P, S], f32, tag="kk")
        va = pers.tile([P, KT_N * 68], f32, tag="va")
        vb = pers.tile([P, KT_N * 68], f32, tag="vb")
        mk = pers.tile([P, 2048], f32, tag="mk")
        abrs = pers.tile([4, KB * P], f32, tag="abrs")
        abjs = pers.tile([4, 512], f32, tag="abjs")
        wosb = pers.tile([P, DM], f32, tag="wosb")
        wqs = pers.tile([P, 8, P], f32, tag="wqs")
        wkvs = pers.tile([P, 8, 256], f32, tag="wkvs")
        idn = pers.tile([P, P], f32, tag="idn")
        onesq = pers.tile([P, P], f32, tag="onesq")
        bqs = pers.tile([P, 1], f32, tag="bqs")
        bkvs = pers.tile([P, 2], f32, tag="bkvs")

        nc.sync.dma_start(
            r(wqs[:]),
            r(wq[:].rearrange("(o p) m -> p o m", p=P)))
        nc.sync.dma_start(idn[:], ident[:])
        absb = pers.tile([P, 256], f32, tag="absb")
        nc.sync.dma_start(bqs[:], bq2[:])
        nc.sync.dma_start(bkvs[:], bkv2[:])
        nc.vector.memset(onesq[:], 0.25)
        # V' ones-columns: memset cannot emit fp32r, so fill a small f32 ones
        # tile and DVE-broadcast-copy it (fp32r-typed out => "rounded" tag)
        onesc = pers.tile([P, 4], f32, tag="onesc")
        nc.vector.memset(onesc[:], 1.0)
        onesqr = pers.tile([P, P], f32, tag="onesqr")
        nc.vector.tensor_copy(r(onesqr[:]), onesq[:])
        for _vall in (va, vb):
            _v3 = _vall[:].rearrange("p (n v) -> p n v", v=68)
            nc.vector.tensor_copy(
                r(_v3[:, :, 64:68]),
                onesc[:, None, :].to_broadcast((P, KT_N, 4)))

        # weights: DMA straight into the operand tiles (the PE reads the
        # fp32 bits in reduced precision; no pre-round pass needed)
        for half in range(2):
            nc.sync.dma_start(
                r(wkvs[:, :, half * P:(half + 1) * P]),
                r(wkv[:, half * P:(half + 1) * P].rearrange(
                    "(o p) m -> p o m", p=P)))

        p1_reps = reps if rep_scope in ("all", "p1") else 1
        p2_reps = reps if rep_scope in ("all", "p2") else 1
        for _rep in range(p1_reps):
            # ---- phase 1: projections, streamed over s
            for hf in range(NHF):
                s0 = hf * QW
                qt8 = qkvp.tile([P, 8, QW], f32, tag="qkvt", name="qkvt8")
                for kt in range(8):
                    nc.sync.dma_start(
                        r(qt8[:, kt, :]),
                        r(qkv_t[kt * P:(kt + 1) * P, s0:s0 + QW]))
                # Q chain first (needs only wqs + slab data -> earliest
                # PE start); V's DVE bias-add runs during the K chain so
                # the transposes at the end never stall the PE
                psq = [sps.tile([P, 512], f32, tag="s", name=f"psq{ci}")
                       for ci in range(NCH)]
                for kt in range(8):
                    for ci in range(NCH):
                        nc.tensor.matmul(
                            psq[ci][:],
                            lhsT=r(wqs[:, kt, :]),
                            rhs=r(qt8[:, kt, ci * 512:(ci + 1) * 512]),
                            start=(kt == 0), stop=(kt == 7))
                psv2 = [sps.tile([P, 512], f32, tag="s", name=f"psv2_{ci}")
                        for ci in range(NCH)]
                for kt in range(8):
                    for ci in range(NCH):
                        nc.tensor.matmul(
                            psv2[ci][:],
                            lhsT=r(wkvs[:, kt, P:256]),
                            rhs=r(qt8[:, kt, ci * 512:(ci + 1) * 512]),
                            start=(kt == 0), stop=(kt == 7))
                for ci in range(NCH):
                    nc.vector.tensor_scalar_add(
                        r(qq[:, s0 + ci * 512:s0 + ci * 512 + 512]),
                        psq[ci][:], bqs[:])
                vsts = []
                for ci in range(NCH):
                    vst = stg.tile([P, 512], f32, tag="vst",
                                   name=f"vst{ci}")
                    nc.vector.tensor_scalar_add(
                        vst[:], psv2[ci][:], bkvs[:, 1:2])
                    vsts.append(vst)
                pskv = [sps.tile([P, 512], f32, tag="s", name=f"pskv{ci}")
                        for ci in range(NCH)]
                for kt in range(8):
                    for ci in range(NCH):
                        nc.tensor.matmul(
                            pskv[ci][:],
                            lhsT=r(wkvs[:, kt, 0:P]),
                            rhs=r(qt8[:, kt, ci * 512:(ci + 1) * 512]),
                            start=(kt == 0), stop=(kt == 7))
                for ci in range(NCH):
                    c0 = s0 + ci * 512
                    for vt in range(4):
                        kt_g = (c0 // P) + vt
                        psv = ops.tile([P, P], f32, tag="o", name=f"psv{vt}")
                        nc.tensor.matmul(
                            psv[:],
                            lhsT=vsts[ci][:, vt * P:(vt + 1) * P],
                            rhs=idn[:],
                            is_transpose=True, start=True, stop=True)
                        nc.vector.tensor_copy(
                            r(va[:, kt_g * 68:kt_g * 68 + 64]), psv[:, 0:64])
                        nc.vector.tensor_copy(
                            r(vb[:, kt_g * 68:kt_g * 68 + 64]), psv[:, 64:P])
                for ci in range(NCH):
                    nc.vector.tensor_scalar_add(
                        r(kk[:, s0 + ci * 512:s0 + ci * 512 + 512]),
                        pskv[ci][:], bkvs[:, 0:1])

            if _rep == 0:
                # phase-2 tables: emitted on the same (sync) DMA queue after
                # every phase-1 slab so they can never delay the qkv stream;
                # they land while phase-1 compute drains, well before use
                nc.sync.dma_start(r(wosb[:]), r(wo[:]))
                nc.sync.dma_start(absb[:], abias[:])
                nc.sync.dma_start(r(abrs[:]), r(abr[:]))
                nc.sync.dma_start(r(abjs[:]), r(abj[:]))
                nc.sync.dma_start(mk[:], masks[:])

        for _rep in range(p2_reps):
            # ---- phase 2: attention + output projection per 512-query chunk
            # Two-deep software pipeline: while chunk qc's pairs stream
            # through QK->exp->PV, the PE slots in chunk qc-1's denominator
            # broadcast and chunk qc-2's output projection right after qc's
            # first pair, so the normalize/stack chains (DVE/Pool/DMA) never
            # gate the PE. Head A (full causal) and head B (banded) pairs
            # are interleaved for the same reason.
            def emit_outproj(stk_t, qc_t):
                for qt in range(4):
                    out_t = outp.tile([P, DM], f32, tag="outt")
                    for nh in range(2):
                        po = pop.tile([P, 512], f32, tag="po")
                        nc.tensor.matmul(
                            po[:],
                            lhsT=r(stk_t[:, qt * P:(qt + 1) * P]),
                            rhs=r(wosb[:, nh * 512:(nh + 1) * 512]),
                            start=True, stop=True)
                        if nh == 0:
                            nc.scalar.copy(
                                out_t[:, nh * 512:(nh + 1) * 512], po[:])
                        else:
                            nc.vector.tensor_copy(
                                out_t[:, nh * 512:(nh + 1) * 512], po[:])
                    nc.sync.dma_start(
                        out[(qc_t * 4 + qt) * P:(qc_t * 4 + qt + 1) * P, :],
                        out_t[:])

            def emit_normalize(o_all):
                # denominator broadcast + approx reciprocal + per-head
                # normalize, stacked [A; B] on 128 partitions for the
                # 128-deep output projection (head B's normalized tile is
                # DMA-shifted straight into partitions 64:128)
                stk = stkp.tile([P, 512], f32, tag="stk")
                dps = [sps.tile([P, 512], f32, tag="s", name=f"dps{h}")
                       for h in range(2)]
                for h in range(2):
                    nc.tensor.matmul(
                        dps[h][:],
                        lhsT=r(onesqr[64:68, 0:P]),
                        rhs=r(o_all[h][64:68, :]),
                        start=True, stop=True)
                rrs = []
                for h in range(2):
                    rr = rrp.tile([P, 512], f32, tag="rr", name=f"rr{h}")
                    nc.vector.reciprocal(rr[:], dps[h][:])
                    rrs.append(rr)
                nc.vector.tensor_tensor(
                    r(stk[0:64, :]), o_all[0][0:64, :], rrs[0][0:64, :], MUL)
                on1 = onp.tile([64, 512], f32, tag="on1")
                nc.vector.tensor_tensor(
                    r(on1[:]), o_all[1][0:64, :], rrs[1][0:64, :], MUL)
                nc.sync.dma_start(r(stk[64:128, :]), r(on1[:]))
                return stk

            pend_o = None  # previous chunk's [68,512] head outputs
            pend_s = None  # (stk, qc) awaiting output projection
            for qc in range(QC_N):
                kend = 4 * (qc + 1)
                tiles_a = [(0, kt) for kt in range(kend)]
                kt0b = max(0, kend - KB)
                tiles_b = [(1, kt) for kt in range(kt0b, kend)]
                singles = []
                ia = ib = 0
                while ia < len(tiles_a) or ib < len(tiles_b):
                    if ia < len(tiles_a):
                        singles.append(tiles_a[ia])
                        ia += 1
                    if ia < len(tiles_a):
                        singles.append(tiles_a[ia])
                        ia += 1
                    if ib < len(tiles_b):
                        singles.append(tiles_b[ib])
                        ib += 1
                o_ps = [ops.tile([68, 512], f32, tag="o", name=f"o_ps{h}")
                        for h in range(2)]
                first = [True, True]
                for idx, (h, kt) in enumerate(singles):
                    r0, r1 = (0, 64) if h == 0 else (64, P)
                    vall = va if h == 0 else vb
                    a = kt - 4 * qc
                    # diagonal tiles: columns j < 128a are entirely above
                    # the causal diagonal, so the whole QK/bias/exp/PV
                    # chain is narrowed to columns >= 128a and only the
                    # [128,128] triangle block needs a mask add
                    n0 = 128 * a if a > 0 else 0
                    nw = 512 - n0
                    ps = sps.tile([P, 512], f32, tag="s")
                    nc.tensor.matmul(
                        ps[:, n0:512],
                        lhsT=r(kk[r0:r1, kt * P:(kt + 1) * P]),
                        rhs=r(qq[r0:r1,
                                 qc * 512 + n0:(qc + 1) * 512]),
                        start=True, stop=(h == 0))
                    if h == 1:
                        # alibi bias lands via PE accumulation: no extra
                        # cross-engine hop on the QK->exp chain
                        nc.tensor.matmul(
                            ps[:, n0:512],
                            lhsT=r(abrs[:, (a + 1) * P:(a + 2) * P]),
                            rhs=r(abjs[:, n0:512]),
                            start=False, stop=True)
                    if "noTT" in ABL:
                        pass
                    elif a >= 0:
                        # intra-block triangle mask (mk's first block is
                        # exactly the [128,128] upper-triangle -1e30 table)
                        nc.vector.tensor_tensor(
                            ps[:, n0:n0 + P], ps[:, n0:n0 + P],
                            mk[:, 0:P], ADD)
                    pt = ptp.tile([P, 512], f32, tag="pt")
                    if "dveexp" in ABL:
                        nc.vector.tensor_copy(r(pt[:, n0:512]), ps[:, n0:512])
                    elif h == 0:
                        bidx = kt * 8 + qc
                        nc.scalar.activation(
                            r(pt[:, n0:512]), ps[:, n0:512], ExpF,
                            bias=absb[:, bidx:bidx + 1])
                    else:
                        nc.scalar.activation(r(pt[:, n0:512]), ps[:, n0:512],
                                             ExpF)
                    nc.tensor.matmul(
                        o_ps[h][:, n0:512],
                        lhsT=r(vall[:, kt * 68:kt * 68 + 68]),
                        rhs=r(pt[:, n0:512]),
                        start=first[h], stop=(kt == kend - 1))
                    first[h] = False
                    if idx == 0 and pend_s is not None:
                        # the pipelined projection of chunk qc-2: operands
                        # long ready, fills the first tile's exp latency
                        emit_outproj(*pend_s)
                        pend_s = None

                # normalize the previous chunk now: its DVE work lands
                # behind this chunk's pair TTs in the queue, so it never
                # delays a pair's exp
                if pend_o is not None:
                    pend_s = (emit_normalize(pend_o), qc - 1)
                    pend_o = None
                # one [68,512] PSUM->SBUF copy per head on the Pool engine:
                # rows 0:64 are the head output, 64:68 the denominators
                o_all = []
                for h in range(2):
                    t = osbp.tile([68, 512], f32, tag="osb", name=f"osb{h}")
                    nc.vector.tensor_copy(r(t[:]), o_ps[h][:])
                    o_all.append(t)
                pend_o = o_all

            stk_last = emit_normalize(pend_o)
            if pend_s is not None:
                emit_outproj(*pend_s)
            emit_outproj(stk_last, QC_N - 1)

    return nc


def core_heads(c):
    return 8 + c, 7 - c


def make_in_maps(qkv, Wq, bq, Wk, bk, Wv, bv, Wo, bo, slopes, S):
    qkv_t = np.ascontiguousarray(qkv[0].T.astype(np.float32))  # [D, S]
    mkv = np.zeros((P, 2048), np.float32)
    pp = np.arange(P)[:, None]
    ff = np.arange(512)[None, :]
    for a in range(4):
        mkv[:, a * 512:(a + 1) * 512] = np.where(a * P + pp > ff, NEG, 0.0)
    idv = np.eye(P, dtype=np.float32)

    in_maps = []
    for c in range(8):
        hA, hB = core_heads(c)
        gA, gB = hA // 4, hB // 4
        sA, sB = float(slopes[hA]), float(slopes[hB])
        wq_c = np.concatenate(
            [Wq[:, hA * DH:(hA + 1) * DH], Wq[:, hB * DH:(hB + 1) * DH]],
            axis=1) * SCALE
        # K pair on cols 0:128, V pair on cols 128:256 (each [A | B])
        wkv_c = np.concatenate(
            [Wk[:, gA * DH:(gA + 1) * DH], Wk[:, gB * DH:(gB + 1) * DH],
             Wv[:, gA * DH:(gA + 1) * DH], Wv[:, gB * DH:(gB + 1) * DH]],
            axis=1)
        wo_c = np.concatenate(
            [Wo[hA * DH:(hA + 1) * DH, :], Wo[hB * DH:(hB + 1) * DH, :]],
            axis=0)
        # head-A alibi bias table: col idx = kt*8 + qc ->
        # slope_A*(128*kt + p) - slope_A*(512*qc + 511), exact fp32
        ab = np.zeros((P, 256), np.float64)
        ppi = np.arange(P)
        for kt in range(S // 128):
            for qcb in range(S // 512):
                ab[:, kt * 8 + qcb] = sA * (128 * kt + ppi) - sA * (512 * qcb + 511)
        # head-B alibi as rank-2 matmul operands: bias[p,j] = c1[a][p] -
        # slope_B*j with hi/lo splits exact under the PE's f32r rounding
        def tf32(x):
            xi = x.astype(np.float32).view(np.int32)
            xi = (xi + (1 << 12)) & ~((1 << 13) - 1)
            return xi.view(np.float32)
        abr_c = np.zeros((4, KB * P), np.float32)
        for ai, a in enumerate(range(-1, 4)):
            c1 = (sB * (128 * a + np.arange(P))).astype(np.float32)
            hi = tf32(c1)
            abr_c[0, ai * P:(ai + 1) * P] = hi
            abr_c[1, ai * P:(ai + 1) * P] = c1 - hi
        abr_c[2] = 1.0
        abr_c[3] = 1.0
        abj_c = np.ones((4, 512), np.float32)
        mj = (-sB * np.arange(512)).astype(np.float32)
        mj_hi = tf32(mj)
        abj_c[2] = mj_hi
        abj_c[3] = mj - mj_hi
        bq2_c = np.concatenate(
            [bq[hA * DH:(hA + 1) * DH], bq[hB * DH:(hB + 1) * DH]]) * SCALE
        bkv2_c = np.stack([
            np.concatenate([bk[gA * DH:(gA + 1) * DH],
                            bk[gB * DH:(gB + 1) * DH]]),
            np.concatenate([bv[gA * DH:(gA + 1) * DH],
                            bv[gB * DH:(gB + 1) * DH]])], axis=1)
        in_maps.append({
            "qkv_t": qkv_t,
            "wq": np.ascontiguousarray(wq_c, np.float32),
            "wkv": np.ascontiguousarray(wkv_c, np.float32),
            "wo": np.ascontiguousarray(wo_c, np.float32),
            "bq2": np.asarray(bq2_c, np.float32).reshape(P, 1),
            "bkv2": np.ascontiguousarray(bkv2_c, np.float32),
            "masks": mkv, "ident": idv,
            "abias": ab.astype(np.float32),
            "abr": abr_c, "abj": abj_c,
        })
    return in_maps


_NC_CACHE = {}


def get_program(S):
    if S not in _NC_CACHE:
        _NC_CACHE[S] = build_program(S)
    return _NC_CACHE[S]


def kernel(qkv, Wq, bq, Wk, bk, Wv, bv, Wo, bo, slopes):
    # the axon NTFF trace path is broken in this container (antenv.axon_hooks
    # missing); make sure a stray BASS_TRACE can never route us into it
    os.environ["BASS_NEVER_TRACE"] = "1"
    qkv = np.asarray(qkv)
    B, S, D = qkv.shape
    args = [np.asarray(x) for x in (Wq, bq, Wk, bk, Wv, bv, Wo, bo, slopes)]
    nc = get_program(S)
    in_maps = make_in_maps(qkv, *args, S=S)
    res = run_bass_kernel_spmd(nc, in_maps, list(range(8)), trace=False)
    LAST["res"] = res
    LAST["exec_time_ns"] = res.exec_time_ns
    partials = np.stack([res.results[c]["out"] for c in range(8)])
    full = partials.sum(axis=0, dtype=np.float64) + np.asarray(bo)
    return full.astype(np.float32).reshape(B, S, D)


# revision 50
# speedup vs baseline: 1.1611x; 1.1611x over previous
"""
Causal ALiBi GQA attention (B=1, S=4096, D=1024, H=16, KVH=4, dh=64) on 8
Trainium2 NeuronCores via Bass/Tile.

Sharding: head-parallel with ALiBi-band load balancing. Core c handles
  - head A = 8+c (small ALiBi slope -> full causal window), and
  - head B = 7-c (large slope -> only the last 6 key-tiles per query chunk
    matter; dropped keys contribute < 1e-9 relative).
Every core therefore runs the identical instruction schedule (SPMD), while
all per-core identity (which heads / kv-heads / slopes) lives in the input
arrays. The 8 partial [S,D] outputs are summed on the host (the unshard).

Device layout (per core), fp32 storage with float32r (single-pass PE,
4x faster than fp32's hi/lo 2-pass) matmuls everywhere:
  - qkv arrives pre-transposed from the host: qkv_t [D, S] (D on
    partitions), DMA'd straight into the matmul operand tiles (PE reads
    the fp32 bits in reduced precision; no pre-round pass needed).
  - Q/K projections emit both heads stacked on 128 partitions (A on 0:64,
    B on 64:128) from one matmul chain; V is projected the same way and
    PE-transposed in [128,128] blocks feeding both heads' V' tiles.
  - Head A: pure q.k fp32r, contraction 64; its alibi enters as an exact
    per-(k-tile, q-chunk) fp32 ACT bias slope_A*(k - q_max(qc)) on the
    exp. The induced per-q factor exp(slope_A*(q - q_max)) cancels in the
    softmax division and stays in fp32 range because slope_A <= 0.075.
  - Head B: pure q.k fp32r as well; its alibi + causal mask come from six
    precomputed [128,512] bias tables indexed by the tile diagonal offset
    a = kt - 4*qc (exact fp32 DVE add before the exp). Logits <= ~3, so
    no running max is needed.
  - Causal mask for head A: -1e30 added on diagonal blocks before exp.
  - k-tiles are processed in pairs sharing one [128,1024] PSUM tile so
    mask/bias adds and head-B exps run at 1024 width.
  - V'_g [128 kpos, 68]: cols 0:64 = V, cols 64:68 = 1.0; PV accumulates
    O' [68, 512q] whose rows 64:68 hold the softmax denominator d[q]. A
    contraction-4 matmul broadcasts d across partitions; after an approx
    reciprocal (18-bit, ample for the 2e-2 gate) + multiply, the two
    normalized heads are stacked [128, 512] so the output projection runs
    with a full 128-deep contraction. PSUM->SBUF output copies run on the
    otherwise-idle Pool engine.
"""

import os
import sys
from contextlib import ExitStack

sys.path.insert(0, "/opt/trn_rl_repo")

import numpy as np

import concourse.bass as bass
import concourse.mybir as mybir
import concourse.tile as tile
from concourse import bass2jax as _bass2jax
from concourse import bass_utils as _bass_utils
from concourse.bass_utils import run_bass_kernel_spmd


def _legalize_bir_sync(bir_json):
    """The TPB ISA embeds at most ONE semaphore wait per instruction
    (NEURON_ISA_TPB_EVENTS has a single wait slot), and this walrus build
    refuses instructions carrying more ("Too many sync wait commands")
    instead of splitting them. Tile attaches up to ~11 waits to one
    instruction, so hoist all but the last wait onto standalone
    EventSemaphore instructions (the exact form raw-bass wait_ge emits)
    immediately before the instruction in its engine stream."""
    import json as _json
    d = _json.loads(bir_json)
    n = 0
    for f in d.get("functions", []):
        for b in f.get("blocks", []):
            insts = b.get("instructions")
            if not insts:
                continue
            out = []
            changed = False
            for i in insts:
                si = i.get("sync_info")
                if si:
                    w = si.get("on_wait") or []
                    u = si.get("on_update") or []
                    assert len(u) <= 1, f"multi-update on {i.get('name')}"
                    if len(w) > 1:
                        changed = True
                        for extra in w[:-1]:
                            n += 1
                            out.append({
                                "debug": i.get("debug", 0),
                                "engine": i["engine"],
                                "ins": [], "outs": [],
                                "name": f"I-legw{n}",
                                "opcode": "EventSemaphore",
                                "sync_info": {"on_update": [],
                                              "on_wait": [extra]},
                            })
                        si["on_wait"] = [w[-1]]
                out.append(i)
            if changed:
                b["instructions"] = out
    return _json.dumps(d).encode()


_ORIG_COMPILE_BIR = _bass_utils.compile_bir_kernel


def _patched_compile_bir_kernel(bir_json, tmpdir, neff_name="file.neff"):
    return _ORIG_COMPILE_BIR(_legalize_bir_sync(bir_json), tmpdir, neff_name)


if _bass_utils.compile_bir_kernel is not _patched_compile_bir_kernel:
    _bass_utils.compile_bir_kernel = _patched_compile_bir_kernel
    _bass2jax.compile_bir_kernel = _patched_compile_bir_kernel

P = 128
DM = 1024
DH = 64
SCALE = 1.0 / 8.0  # 1/sqrt(dh)
NEG = -1.0e30
KB = 5  # banded head: key-tiles kept per query chunk; nearest dropped
        # key sits >= 128 positions behind the chunk, weight < 2e-6

LAST = {}
ABL = set()


def build_program(S, reps=1, rep_scope="all"):
    f32 = mybir.dt.float32
    f32r = mybir.dt.float32r

    def r(ap):
        # single-pass reduced-precision PE multiply: 4x faster than fp32
        return ap.bitcast(f32r)
    KT_N = S // 128
    QC_N = S // 512

    nc = bass.Bass()
    qkv_t = nc.dram_tensor("qkv_t", [DM, S], f32, kind="ExternalInput")
    wq = nc.dram_tensor("wq", [DM, P], f32, kind="ExternalInput")
    wkv = nc.dram_tensor("wkv", [DM, 256], f32, kind="ExternalInput")
    wo = nc.dram_tensor("wo", [P, DM], f32, kind="ExternalInput")
    bq2 = nc.dram_tensor("bq2", [P, 1], f32, kind="ExternalInput")
    bkv2 = nc.dram_tensor("bkv2", [P, 2], f32, kind="ExternalInput")
    abias = nc.dram_tensor("abias", [P, 256], f32, kind="ExternalInput")
    # post-exp mask tables: tri is the [128,128] 0/1 lower-triangle; ebt
    # holds exp(slope_B*(k-q)) per diagonal offset a (0 above the diagonal),
    # so one multiply applies head B's alibi AND its causal mask
    ebt = nc.dram_tensor("ebt", [P, KB * 512], f32, kind="ExternalInput")
    tri = nc.dram_tensor("tri", [P, P], f32, kind="ExternalInput")
    ident = nc.dram_tensor("ident", [P, P], f32, kind="ExternalInput")
    out = nc.dram_tensor("out", [S, DM], f32, kind="ExternalOutput")

    ExpF = mybir.ActivationFunctionType.Exp
    ADD = mybir.AluOpType.add
    MUL = mybir.AluOpType.mult

    QW = 1024 if S >= 1024 else S  # s-stream width for projections
    NHF = S // QW
    NCH = QW // 512

    with ExitStack() as ctx:
        tc = ctx.enter_context(tile.TileContext(nc))
        pers = ctx.enter_context(tc.tile_pool(name="pers", bufs=1))
        qkvp = ctx.enter_context(tc.tile_pool(name="qkvp", bufs=2))
        stg = ctx.enter_context(tc.tile_pool(name="stg", bufs=2))
        ptp = ctx.enter_context(tc.tile_pool(name="ptp", bufs=6))
        osbp = ctx.enter_context(tc.tile_pool(name="osbp", bufs=3))
        rrp = ctx.enter_context(tc.tile_pool(name="rrp", bufs=2))
        onp = ctx.enter_context(tc.tile_pool(name="onp", bufs=2))
        stkp = ctx.enter_context(tc.tile_pool(name="stkp", bufs=2))
        outp = ctx.enter_context(tc.tile_pool(name="outp", bufs=4))
        sps = ctx.enter_context(tc.tile_pool(name="sps", bufs=4, space="PSUM"))
        pop = ctx.enter_context(tc.tile_pool(name="pop", bufs=2, space="PSUM"))
        ops = ctx.enter_context(tc.tile_pool(name="ops", bufs=2, space="PSUM"))

        qq = pers.tile([P, S], f32, tag="qq")
        kk = pers.tile([P, S], f32, tag="kk")
        va = pers.tile([P, KT_N * 68], f32, tag="va")
        vb = pers.tile([P, KT_N * 68], f32, tag="vb")
        ebts = pers.tile([P, KB * 512], f32, tag="ebts")
        tris = pers.tile([P, P], f32, tag="tris")
        wosb = pers.tile([P, DM], f32, tag="wosb")
        wqs = pers.tile([P, 8, P], f32, tag="wqs")
        wkvs = pers.tile([P, 8, 256], f32, tag="wkvs")
        idn = pers.tile([P, P], f32, tag="idn")
        onesq = pers.tile([P, P], f32, tag="onesq")
        bqs = pers.tile([P, 1], f32, tag="bqs")
        bkvs = pers.tile([P, 2], f32, tag="bkvs")

        nc.sync.dma_start(
            r(wqs[:]),
            r(wq[:].rearrange("(o p) m -> p o m", p=P)))
        nc.sync.dma_start(idn[:], ident[:])
        absb = pers.tile([P, 256], f32, tag="absb")
        nc.sync.dma_start(bqs[:], bq2[:])
        nc.sync.dma_start(bkvs[:], bkv2[:])
        nc.vector.memset(onesq[:], 0.25)
        # V' ones-columns: memset cannot emit fp32r, so fill a small f32 ones
        # tile and DVE-broadcast-copy it (fp32r-typed out => "rounded" tag)
        onesc = pers.tile([P, 4], f32, tag="onesc")
        nc.vector.memset(onesc[:], 1.0)
        onesqr = pers.tile([P, P], f32, tag="onesqr")
        nc.vector.tensor_copy(r(onesqr[:]), onesq[:])
        for _vall in (va, vb):
            _v3 = _vall[:].rearrange("p (n v) -> p n v", v=68)
            nc.vector.tensor_copy(
                r(_v3[:, :, 64:68]),
                onesc[:, None, :].to_broadcast((P, KT_N, 4)))

        # weights: DMA straight into the operand tiles (the PE reads the
        # fp32 bits in reduced precision; no pre-round pass needed)
        for half in range(2):
            nc.sync.dma_start(
                r(wkvs[:, :, half * P:(half + 1) * P]),
                r(wkv[:, half * P:(half + 1) * P].rearrange(
                    "(o p) m -> p o m", p=P)))

        p1_reps = reps if rep_scope in ("all", "p1") else 1
        p2_reps = reps if rep_scope in ("all", "p2") else 1
        for _rep in range(p1_reps):
            # ---- phase 1: projections, streamed over s
            for hf in range(NHF):
                s0 = hf * QW
                qt8 = qkvp.tile([P, 8, QW], f32, tag="qkvt", name="qkvt8")
                for kt in range(8):
                    nc.sync.dma_start(
                        r(qt8[:, kt, :]),
                        r(qkv_t[kt * P:(kt + 1) * P, s0:s0 + QW]))
                # Q chain first (needs only wqs + slab data -> earliest
                # PE start); V's DVE bias-add runs during the K chain so
                # the transposes at the end never stall the PE
                psq = [sps.tile([P, 512], f32, tag="s", name=f"psq{ci}")
                       for ci in range(NCH)]
                for kt in range(8):
                    for ci in range(NCH):
                        nc.tensor.matmul(
                            psq[ci][:],
                            lhsT=r(wqs[:, kt, :]),
                            rhs=r(qt8[:, kt, ci * 512:(ci + 1) * 512]),
                            start=(kt == 0), stop=(kt == 7))
                psv2 = [sps.tile([P, 512], f32, tag="s", name=f"psv2_{ci}")
                        for ci in range(NCH)]
                for kt in range(8):
                    for ci in range(NCH):
                        nc.tensor.matmul(
                            psv2[ci][:],
                            lhsT=r(wkvs[:, kt, P:256]),
                            rhs=r(qt8[:, kt, ci * 512:(ci + 1) * 512]),
                            start=(kt == 0), stop=(kt == 7))
                for ci in range(NCH):
                    nc.vector.tensor_scalar_add(
                        r(qq[:, s0 + ci * 512:s0 + ci * 512 + 512]),
                        psq[ci][:], bqs[:])
                vsts = []
                for ci in range(NCH):
                    vst = stg.tile([P, 512], f32, tag="vst",
                                   name=f"vst{ci}")
                    nc.vector.tensor_scalar_add(
                        vst[:], psv2[ci][:], bkvs[:, 1:2])
                    vsts.append(vst)
                pskv = [sps.tile([P, 512], f32, tag="s", name=f"pskv{ci}")
                        for ci in range(NCH)]
                for kt in range(8):
                    for ci in range(NCH):
                        nc.tensor.matmul(
                            pskv[ci][:],
                            lhsT=r(wkvs[:, kt, 0:P]),
                            rhs=r(qt8[:, kt, ci * 512:(ci + 1) * 512]),
                            start=(kt == 0), stop=(kt == 7))
                for ci in range(NCH):
                    c0 = s0 + ci * 512
                    for vt in range(4):
                        kt_g = (c0 // P) + vt
                        psv = ops.tile([P, P], f32, tag="o", name=f"psv{vt}")
                        nc.tensor.matmul(
                            psv[:],
                            lhsT=vsts[ci][:, vt * P:(vt + 1) * P],
                            rhs=idn[:],
                            is_transpose=True, start=True, stop=True)
                        nc.vector.tensor_copy(
                            r(va[:, kt_g * 68:kt_g * 68 + 64]), psv[:, 0:64])
                        nc.vector.tensor_copy(
                            r(vb[:, kt_g * 68:kt_g * 68 + 64]), psv[:, 64:P])
                for ci in range(NCH):
                    nc.vector.tensor_scalar_add(
                        r(kk[:, s0 + ci * 512:s0 + ci * 512 + 512]),
                        pskv[ci][:], bkvs[:, 0:1])

            if _rep == 0:
                # phase-2 tables: emitted on the same (sync) DMA queue after
                # every phase-1 slab so they can never delay the qkv stream;
                # they land while phase-1 compute drains, well before use
                nc.sync.dma_start(r(wosb[:]), r(wo[:]))
                nc.sync.dma_start(absb[:], abias[:])
                nc.sync.dma_start(ebts[:], ebt[:])
                nc.sync.dma_start(tris[:], tri[:])

        for _rep in range(p2_reps):
            # ---- phase 2: attention + output projection per 512-query chunk
            # Two-deep software pipeline: while chunk qc's pairs stream
            # through QK->exp->PV, the PE slots in chunk qc-1's denominator
            # broadcast and chunk qc-2's output projection right after qc's
            # first pair, so the normalize/stack chains (DVE/Pool/DMA) never
            # gate the PE. Head A (full causal) and head B (banded) pairs
            # are interleaved for the same reason.
            def emit_outproj(stk_t, qc_t):
                for qt in range(4):
                    out_t = outp.tile([P, DM], f32, tag="outt")
                    for nh in range(2):
                        po = pop.tile([P, 512], f32, tag="po")
                        nc.tensor.matmul(
                            po[:],
                            lhsT=r(stk_t[:, qt * P:(qt + 1) * P]),
                            rhs=r(wosb[:, nh * 512:(nh + 1) * 512]),
                            start=True, stop=True)
                        if nh == 0:
                            nc.scalar.copy(
                                out_t[:, nh * 512:(nh + 1) * 512], po[:])
                        else:
                            nc.vector.tensor_copy(
                                out_t[:, nh * 512:(nh + 1) * 512], po[:])
                    nc.sync.dma_start(
                        out[(qc_t * 4 + qt) * P:(qc_t * 4 + qt + 1) * P, :],
                        out_t[:])

            def emit_normalize(o_all):
                # denominator broadcast + approx reciprocal + per-head
                # normalize, stacked [A; B] on 128 partitions for the
                # 128-deep output projection (head B's normalized tile is
                # DMA-shifted straight into partitions 64:128)
                stk = stkp.tile([P, 512], f32, tag="stk")
                dps = [sps.tile([P, 512], f32, tag="s", name=f"dps{h}")
                       for h in range(2)]
                for h in range(2):
                    nc.tensor.matmul(
                        dps[h][:],
                        lhsT=r(onesqr[64:68, 0:P]),
                        rhs=r(o_all[h][64:68, :]),
                        start=True, stop=True)
                rrs = []
                for h in range(2):
                    rr = rrp.tile([P, 512], f32, tag="rr", name=f"rr{h}")
                    nc.vector.reciprocal(rr[:], dps[h][:])
                    rrs.append(rr)
                nc.vector.tensor_tensor(
                    r(stk[0:64, :]), o_all[0][0:64, :], rrs[0][0:64, :], MUL)
                on1 = onp.tile([64, 512], f32, tag="on1")
                nc.vector.tensor_tensor(
                    r(on1[:]), o_all[1][0:64, :], rrs[1][0:64, :], MUL)
                nc.sync.dma_start(r(stk[64:128, :]), r(on1[:]))
                return stk

            pend_o = None  # previous chunk's [68,512] head outputs
            pend_s = None  # (stk, qc) awaiting output projection
            for qc in range(QC_N):
                kend = 4 * (qc + 1)
                tiles_a = [(0, kt) for kt in range(kend)]
                kt0b = max(0, kend - KB)
                tiles_b = [(1, kt) for kt in range(kt0b, kend)]
                singles = []
                ia = ib = 0
                while ia < len(tiles_a) or ib < len(tiles_b):
                    if ia < len(tiles_a):
                        singles.append(tiles_a[ia])
                        ia += 1
                    if ia < len(tiles_a):
                        singles.append(tiles_a[ia])
                        ia += 1
                    if ib < len(tiles_b):
                        singles.append(tiles_b[ib])
                        ib += 1
                o_ps = [ops.tile([68, 512], f32, tag="o", name=f"o_ps{h}")
                        for h in range(2)]
                first = [True, True]
                for idx, (h, kt) in enumerate(singles):
                    r0, r1 = (0, 64) if h == 0 else (64, P)
                    vall = va if h == 0 else vb
                    a = kt - 4 * qc
                    # diagonal tiles: columns j < 128a are entirely above
                    # the causal diagonal, so the whole QK/exp/PV chain is
                    # narrowed to columns >= 128a; masking happens post-exp
                    # on the otherwise-idle Pool engine
                    n0 = 128 * a if a > 0 else 0
                    ps = sps.tile([P, 512], f32, tag="s")
                    nc.tensor.matmul(
                        ps[:, n0:512],
                        lhsT=r(kk[r0:r1, kt * P:(kt + 1) * P]),
                        rhs=r(qq[r0:r1,
                                 qc * 512 + n0:(qc + 1) * 512]),
                        start=True, stop=True)
                    pt = ptp.tile([P, 512], f32, tag="pt")
                    if "dveexp" in ABL:
                        nc.vector.tensor_copy(r(pt[:, n0:512]), ps[:, n0:512])
                    elif h == 0:
                        bidx = kt * 8 + qc
                        nc.scalar.activation(
                            r(pt[:, n0:512]), ps[:, n0:512], ExpF,
                            bias=absb[:, bidx:bidx + 1])
                    else:
                        nc.scalar.activation(r(pt[:, n0:512]), ps[:, n0:512],
                                             ExpF)
                    if "noTT" in ABL:
                        pass
                    elif h == 0:
                        if a >= 0:
                            # zero the above-diagonal triangle block
                            nc.gpsimd.tensor_tensor(
                                r(pt[:, n0:n0 + P]), pt[:, n0:n0 + P],
                                tris[:], MUL)
                    else:
                        # alibi decay + causal mask in one multiply
                        nc.gpsimd.tensor_tensor(
                            r(pt[:, n0:512]), pt[:, n0:512],
                            ebts[:, (a + 1) * 512 + n0:(a + 2) * 512], MUL)
                    nc.tensor.matmul(
                        o_ps[h][:, n0:512],
                        lhsT=r(vall[:, kt * 68:kt * 68 + 68]),
                        rhs=r(pt[:, n0:512]),
                        start=first[h], stop=(kt == kend - 1))
                    first[h] = False
                    if idx == 0 and pend_s is not None:
                        # the pipelined projection of chunk qc-2: operands
                        # long ready, fills the first tile's exp latency
                        emit_outproj(*pend_s)
                        pend_s = None

                # normalize the previous chunk now: its DVE work lands
                # behind this chunk's pair TTs in the queue, so it never
                # delays a pair's exp
                if pend_o is not None:
                    pend_s = (emit_normalize(pend_o), qc - 1)
                    pend_o = None
                # one [68,512] PSUM->SBUF copy per head on the Pool engine:
                # rows 0:64 are the head output, 64:68 the denominators
                o_all = []
                for h in range(2):
                    t = osbp.tile([68, 512], f32, tag="osb", name=f"osb{h}")
                    nc.vector.tensor_copy(r(t[:]), o_ps[h][:])
                    o_all.append(t)
                pend_o = o_all

            stk_last = emit_normalize(pend_o)
            if pend_s is not None:
                emit_outproj(*pend_s)
            emit_outproj(stk_last, QC_N - 1)

    return nc


def core_heads(c):
    return 8 + c, 7 - c


def make_in_maps(qkv, Wq, bq, Wk, bk, Wv, bv, Wo, bo, slopes, S):
    qkv_t = np.ascontiguousarray(qkv[0].T.astype(np.float32))  # [D, S]
    pp = np.arange(P)[:, None]
    triv = (pp <= np.arange(P)[None, :]).astype(np.float32)
    idv = np.eye(P, dtype=np.float32)

    in_maps = []
    for c in range(8):
        hA, hB = core_heads(c)
        gA, gB = hA // 4, hB // 4
        sA, sB = float(slopes[hA]), float(slopes[hB])
        wq_c = np.concatenate(
            [Wq[:, hA * DH:(hA + 1) * DH], Wq[:, hB * DH:(hB + 1) * DH]],
            axis=1) * SCALE
        # K pair on cols 0:128, V pair on cols 128:256 (each [A | B])
        wkv_c = np.concatenate(
            [Wk[:, gA * DH:(gA + 1) * DH], Wk[:, gB * DH:(gB + 1) * DH],
             Wv[:, gA * DH:(gA + 1) * DH], Wv[:, gB * DH:(gB + 1) * DH]],
            axis=1)
        wo_c = np.concatenate(
            [Wo[hA * DH:(hA + 1) * DH, :], Wo[hB * DH:(hB + 1) * DH, :]],
            axis=0)
        # head-A alibi bias table: col idx = kt*8 + qc ->
        # slope_A*(128*kt + p) - slope_A*(512*qc + 511), exact fp32
        ab = np.zeros((P, 256), np.float64)
        ppi = np.arange(P)
        for kt in range(S // 128):
            for qcb in range(S // 512):
                ab[:, kt * 8 + qcb] = sA * (128 * kt + ppi) - sA * (512 * qcb + 511)
        # head-B post-exp tables: exp(slope_B*(k-q)) per diagonal offset
        # a = kt - 4*qc in {-1..3}, zero above the causal diagonal
        ebt_c = np.zeros((P, KB * 512), np.float32)
        jj = np.arange(512)[None, :]
        for ai, a in enumerate(range(-1, 4)):
            kq = 128 * a + pp - jj
            ebt_c[:, ai * 512:(ai + 1) * 512] = np.where(
                kq <= 0, np.exp(sB * np.minimum(kq, 0.0)), 0.0)
        bq2_c = np.concatenate(
            [bq[hA * DH:(hA + 1) * DH], bq[hB * DH:(hB + 1) * DH]]) * SCALE
        bkv2_c = np.stack([
            np.concatenate([bk[gA * DH:(gA + 1) * DH],
                            bk[gB * DH:(gB + 1) * DH]]),
            np.concatenate([bv[gA * DH:(gA + 1) * DH],
                            bv[gB * DH:(gB + 1) * DH]])], axis=1)
        in_maps.append({
            "qkv_t": qkv_t,
            "wq": np.ascontiguousarray(wq_c, np.float32),
            "wkv": np.ascontiguousarray(wkv_c, np.float32),
            "wo": np.ascontiguousarray(wo_c, np.float32),
            "bq2": np.asarray(bq2_c, np.float32).reshape(P, 1),
            "bkv2": np.ascontiguousarray(bkv2_c, np.float32),
            "ident": idv,
            "abias": ab.astype(np.float32),
            "ebt": ebt_c, "tri": triv,
        })
    return in_maps


_NC_CACHE = {}


def get_program(S):
    if S not in _NC_CACHE:
        _NC_CACHE[S] = build_program(S)
    return _NC_CACHE[S]


def kernel(qkv, Wq, bq, Wk, bk, Wv, bv, Wo, bo, slopes):
    # the axon NTFF trace path is broken in this container (antenv.axon_hooks
    # missing); make sure a stray BASS_TRACE can never route us into it
    os.environ["BASS_NEVER_TRACE"] = "1"
    qkv = np.asarray(qkv)
    B, S, D = qkv.shape
    args = [np.asarray(x) for x in (Wq, bq, Wk, bk, Wv, bv, Wo, bo, slopes)]
    nc = get_program(S)
    in_maps = make_in_maps(qkv, *args, S=S)
    res = run_bass_kernel_spmd(nc, in_maps, list(range(8)), trace=False)
    LAST["res"] = res
    LAST["exec_time_ns"] = res.exec_time_ns
    partials = np.stack([res.results[c]["out"] for c in range(8)])
    full = partials.sum(axis=0, dtype=np.float64) + np.asarray(bo)
    return full.astype(np.float32).reshape(B, S, D)
